# revision 12
# baseline (speedup 1.0000x reference)
"""MDCA loss kernel for Trainium2 (8 NeuronCores, SPMD data-parallel).

Problem: 4 CAMs [128, 1000, 14, 14] f32 + target [128] i64 ->
4 scalar losses: mean_c |mean_{b,h,w} cam[b,c,h,w] - bincount(target)[c]/B|.

Strategy (memory-bound, ~401 MB total input, HW-measured ~100 us/core):
  - Shard batch across 8 cores: 16 rows/core, ~50 MB/core.
  - "flat" layout (default): view each core's contiguous [16, 196000] cam
    shard as [(row eighth)=128p, 24500]; partition p = 8*row + e holds
    classes [125e, 125(e+1)) of row p//8. Each DMA loads a class-aligned
    free-axis chunk [128, (125/FLAT_Q)*196] -> ONE contiguous ~19.6 KB
    descriptor per partition (descriptor size dominates HW DMA rate:
    6 KB descs measure ~170 GB/s/core, 19.6 KB ~510 GB/s). DVE
    reduce_sum collapses each tile's hw axis -> per-(partition, class)
    sums; no cross-partition reduce needed on-device.
  - Loads stream on the sync HWDGE ring; the [128, 500] result DMA runs
    on the scalar ring so the loader never blocks; FLAT_NBUFS slots
    double-buffer DMA against DVE (DVE ~76 us hides under DMA ~98 us).
  - Host sums the 8 cores' [128, 500] partials -> per-class totals,
    adds bincount(target)/B, computes the 4 scalar losses in f64.
  - The older "rows" layout (_build_nc) is kept for reference; it is
    ~3x slower on HW (295 us) despite identical cost-model estimates.

Raw Bass Block (not Tile): HWDGE DMA instructions only support one inline
sync-wait, so semaphores are placed by hand — one completion sem per SBUF
slot (concurrent DMAs always target distinct slots), WAR on slot reuse
guarded transitively through the DVE sem.
"""

import numpy as np

B, C, H, W = 128, 1000, 14, 14
HWSZ = H * W                 # 196
N_CORES = 8
B_SH = B // N_CORES          # 16 batch rows per core
P = 125                      # partitions used; class c -> (p=c//8, cc=c%8)
CC = 8                       # classes per partition
RUN = CC * HWSZ              # 1568 contiguous f32 per (p, b)
F = C * HWSZ                 # 196000 elements per batch row
N_CAMS = 4

CHUNK_B = 1                  # batch rows per load tile
N_BUFS = 12                  # SBUF slots (CHUNK_B*6272 B/partition each)
DUAL_RING = False            # issue loads alternately from sync and scalar HWDGE

# flat layout: view the [16, 196000] shard as [(b e)=128, 24500]; partition
# p = 8*row + e holds classes [125e, 125(e+1)) of row p//8 — per-partition
# DMA runs are 24500/FLAT_Q contiguous f32, one descriptor per partition.
LAYOUT = "flat"              # "rows" (_build_nc) or "flat" (_build_nc_flat)
FLAT_Q = 5                   # chunks per cam (1, 5, or 25: class-aligned)
FLAT_NBUFS = 10
XP = 24500                   # elems per partition per cam (125 classes)
PC = 125                     # classes per partition per cam

_CACHE = {}


def _build_nc_flat(q=None, n_bufs=None, n_iters=1, dual_ring=None,
                   ncls_chunk=None):
    from contextlib import ExitStack

    import concourse.bass as bass
    import concourse.mybir as mybir

    q = FLAT_Q if q is None else q
    nb = FLAT_NBUFS if n_bufs is None else n_bufs
    dual = DUAL_RING if dual_ring is None else dual_ring
    if ncls_chunk is None:
        assert PC % q == 0
        ncls_chunk = PC // q     # classes per chunk per partition
    # class-aligned chunks per cam; last chunk may be partial
    chunks = [(o, min(ncls_chunk, PC - o)) for o in range(0, PC, ncls_chunk)]
    cs = ncls_chunk * HWSZ       # buffer elems per partition (max chunk)
    n_loads = N_CAMS * len(chunks)  # loads (== DVE ops) per iteration

    f32 = mybir.dt.float32
    nc = bass.Bass()
    cams = [
        nc.dram_tensor(f"cam_{i}", [B_SH, F], f32, kind="ExternalInput")
        for i in range(N_CAMS)
    ]
    out = nc.dram_tensor("sums", [128, N_CAMS * PC], f32, kind="ExternalOutput")

    with ExitStack() as ctx:
        bufs = [
            ctx.enter_context(nc.sbuf_tensor(f"t{s}", [128, cs], f32))
            for s in range(nb)
        ]
        # double-buffered result tile: DVE writes osum[g%2] while the scalar
        # ring drains osum[(g-1)%2] to DRAM
        osums = [
            ctx.enter_context(nc.sbuf_tensor(f"osum{s}", [128, N_CAMS * PC], f32))
            for s in range(2)
        ]
        slot_sems = [
            ctx.enter_context(nc.semaphore(f"slot_sem{s}")) for s in range(nb)
        ]
        out_sem = ctx.enter_context(nc.semaphore("out_sem"))
        dve_sem = ctx.enter_context(nc.semaphore("dve_sem"))
        block = ctx.enter_context(nc.Block())

        def loader(eng, g, parity):
            for n in range(n_loads):
                if parity is not None and n % 2 != parity:
                    continue
                i, j = divmod(n, len(chunks))
                off, nck = chunks[j]
                ce = nck * HWSZ
                gn = g * n_loads + n
                if gn >= nb:
                    # slot's previous tile consumed by its DVE reduce
                    eng.wait_ge(dve_sem, gn - nb + 1)
                src = cams[i].rearrange("b (e x) -> (b e) x", e=8, x=XP)[
                    :, off * HWSZ:off * HWSZ + ce
                ]
                eng.dma_start(bufs[gn % nb][:, :ce], src).then_inc(
                    slot_sems[gn % nb], 16
                )

        @block.sync
        def _(sync):
            for g in range(n_iters):
                loader(sync, g, 0 if dual else None)
            sync.wait_ge(out_sem, 16 * n_iters)

        @block.scalar
        def _(scalar):
            for g in range(n_iters):
                if dual:
                    loader(scalar, g, 1)
                scalar.wait_ge(dve_sem, (g + 1) * n_loads)
                scalar.dma_start(out[:, :], osums[g % 2][:]).then_inc(out_sem, 16)

        @block.vector
        def _(vector):
            for g in range(n_iters):
                for n in range(n_loads):
                    i, j = divmod(n, len(chunks))
                    off, nck = chunks[j]
                    gn = g * n_loads + n
                    if g > 1 and n == 0:
                        # WAR: osum[g%2] reread by iteration g-2's out DMA
                        vector.wait_ge(out_sem, 16 * (g - 1))
                    vector.wait_ge(slot_sems[gn % nb], 16 * (gn // nb + 1))
                    nc.vector.reduce_sum(
                        out=osums[g % 2][
                            :, i * PC + off:i * PC + off + nck
                        ],
                        in_=bufs[gn % nb][:, :nck * HWSZ].rearrange(
                            "p (c x) -> p c x", c=nck
                        ),
                        axis=mybir.AxisListType.X,
                    ).then_inc(dve_sem, 1)

    return nc


def _build_nc(chunk_b=None, n_bufs=None, n_iters=1, dual_ring=None):
    from contextlib import ExitStack

    import concourse.bass as bass
    import concourse.mybir as mybir

    cb = CHUNK_B if chunk_b is None else chunk_b
    nb = N_BUFS if n_bufs is None else n_bufs
    dual = DUAL_RING if dual_ring is None else dual_ring
    n_chunks = B_SH // cb            # loads per cam
    n_loads = N_CAMS * n_chunks      # loads per iteration
    dve_per_iter = n_loads + N_CAMS  # stage1 + stage2 ops per iteration

    def dve_after_s1(k):
        # dve_sem value right after stage1-reduce #k retires (DVE order per
        # cam: n_chunks * s1 then one s2)
        return k + k // n_chunks + 1

    f32 = mybir.dt.float32
    nc = bass.Bass()
    cams = [
        nc.dram_tensor(f"cam_{i}", [B_SH, F], f32, kind="ExternalInput")
        for i in range(N_CAMS)
    ]
    out = nc.dram_tensor("sums", [P, N_CAMS * CC], f32, kind="ExternalOutput")

    with ExitStack() as ctx:
        bufs = [
            ctx.enter_context(nc.sbuf_tensor(f"t{s}", [P, cb, RUN], f32))
            for s in range(nb)
        ]
        stages = [
            ctx.enter_context(nc.sbuf_tensor(f"stage{i}", [P, n_chunks, cb, CC], f32))
            for i in range(N_CAMS)
        ]
        out_sums = ctx.enter_context(nc.sbuf_tensor("osum", [P, N_CAMS * CC], f32))
        # one completion sem per buffer slot: concurrent loads target distinct
        # slots, so "slot_sem >= 16*k" unambiguously means "k-th load into this
        # slot is fully complete" (each DMA is 16 sub-completions)
        slot_sems = [
            ctx.enter_context(nc.semaphore(f"slot_sem{s}")) for s in range(nb)
        ]
        out_sem = ctx.enter_context(nc.semaphore("out_sem"))
        dve_sem = ctx.enter_context(nc.semaphore("dve_sem"))
        block = ctx.enter_context(nc.Block())

        def loader(eng, g, parity):
            # emit this engine's share of iteration g's loads (all, or
            # odd/even when dual-ring); slot-reuse WAR is guarded via
            # dve_sem transitively
            for n in range(n_loads):
                if parity is not None and n % 2 != parity:
                    continue
                i, c = divmod(n, n_chunks)
                gn = g * n_loads + n
                if gn >= nb:
                    # slot's previous tile fully consumed by its stage1
                    # reduce (which also implies that old DMA completed)
                    pk = gn - nb
                    eng.wait_ge(
                        dve_sem,
                        (pk // n_loads) * dve_per_iter
                        + dve_after_s1(pk % n_loads),
                    )
                src = cams[i][c * cb:(c + 1) * cb, :].rearrange(
                    "b (p x) -> p b x", p=P, x=RUN
                )
                eng.dma_start(bufs[gn % nb][:], src).then_inc(
                    slot_sems[gn % nb], 16
                )

        @block.sync
        def _(sync):
            for g in range(n_iters):
                loader(sync, g, 0 if dual else None)
                sync.wait_ge(dve_sem, (g + 1) * dve_per_iter)
                sync.dma_start(out[:, :], out_sums[:]).then_inc(out_sem, 16)
            sync.wait_ge(out_sem, 16 * n_iters)

        if dual:

            @block.scalar
            def _(scalar):
                for g in range(n_iters):
                    loader(scalar, g, 1)

        @block.vector
        def _(vector):
            for g in range(n_iters):
                dve_base = g * dve_per_iter
                for i in range(N_CAMS):
                    for c in range(n_chunks):
                        n = i * n_chunks + c
                        gn = g * n_loads + n
                        if g > 0 and c == 0:
                            # WAR: stages[i] reread by prev iter's stage2
                            vector.wait_ge(
                                dve_sem,
                                (g - 1) * dve_per_iter
                                + (i + 1) * (n_chunks + 1),
                            )
                        vector.wait_ge(
                            slot_sems[gn % nb], 16 * (gn // nb + 1)
                        )
                        nc.vector.reduce_sum(
                            out=stages[i][:, c],
                            in_=bufs[gn % nb][:].rearrange(
                                "p b (cc xx) -> p b cc xx", cc=CC
                            ),
                            axis=mybir.AxisListType.X,
                        ).then_inc(dve_sem, 1)
                    # reduce the 16 batch partials per class:
                    # [P, cc, (chunks b)] -> [P, cc]; same-engine wait makes
                    # sure the stage1 writes retired before this read
                    vector.wait_ge(dve_sem, dve_base + (i + 1) * n_chunks + i)
                    # WAR vs previous iteration's out DMA
                    if g > 0 and i == 0:
                        vector.wait_ge(out_sem, 16 * g)
                    nc.vector.reduce_sum(
                        out=out_sums[:, i * CC:(i + 1) * CC],
                        in_=stages[i][:].rearrange("p h b cc -> p cc (h b)"),
                        axis=mybir.AxisListType.X,
                    ).then_inc(dve_sem, 1)

    return nc


def _build_bench_nc(n_iters=1):
    return (
        _build_nc_flat(n_iters=n_iters)
        if LAYOUT == "flat"
        else _build_nc(n_iters=n_iters)
    )


def _get_nc():
    if "nc" not in _CACHE:
        _CACHE["nc"] = _build_bench_nc()
    return _CACHE["nc"]


def _run_on_device(in_maps, nc=None, **kwargs):
    from concourse.bass_utils import run_bass_kernel_spmd

    return run_bass_kernel_spmd(
        nc if nc is not None else _get_nc(),
        in_maps,
        core_ids=list(range(N_CORES)),
        **kwargs,
    )


def _make_in_maps(cams):
    in_maps = []
    for k in range(N_CORES):
        m = {}
        for i, cam in enumerate(cams):
            m[f"cam_{i}"] = np.ascontiguousarray(
                np.asarray(cam)[k * B_SH:(k + 1) * B_SH].reshape(B_SH, F),
                dtype=np.float32,
            )
        in_maps.append(m)
    return in_maps


def kernel(cam_0, cam_1, cam_2, cam_3, target, _bench_results=None, **_kw):
    in_maps = _make_in_maps((cam_0, cam_1, cam_2, cam_3))
    res = _run_on_device(in_maps)
    if _bench_results is not None:
        _bench_results.append(res)

    # host combine: per-core partials -> per-class totals -> scalar losses
    counts = np.bincount(np.asarray(target).astype(np.int64), minlength=C)
    avg_count = counts.astype(np.float64) / B

    per_class_tot = np.zeros((N_CAMS, C), dtype=np.float64)
    if LAYOUT == "flat":
        # [128, 4*125] per core; partition p = 8*row + e covers classes
        # [125e, 125(e+1)) of that row -> reshape [16, 8*125=1000], sum rows
        for r in res.results:
            s = r["sums"].astype(np.float64)
            for i in range(N_CAMS):
                per_class_tot[i] += (
                    s[:, i * PC:(i + 1) * PC].reshape(B_SH, C).sum(axis=0)
                )
    else:
        total = np.zeros((P, N_CAMS * CC), dtype=np.float64)
        for r in res.results:
            total += r["sums"].astype(np.float64)
        for i in range(N_CAMS):
            per_class_tot[i] = total[:, i * CC:(i + 1) * CC].reshape(C)

    losses = []
    for i in range(N_CAMS):
        avg_conf = per_class_tot[i] / (B * HWSZ)
        losses.append(np.float32(np.abs(avg_conf - avg_count).mean()))
    return tuple(np.asarray(l, dtype=np.float32) for l in losses)

